# revision 7
# baseline (speedup 1.0000x reference)
"""CutoutColor Trainium2 kernel.

out[n,c,h,w] = colors[n,c] if (tops[n] <= h < tops[n]+28 and
                               lefts[n] <= w < lefts[n]+28) else x[n,c,h,w]

Strategy: pure data parallel over the batch axis, 512 samples per core on
8 NeuronCores.  On each core, samples are processed in 4 groups of 128
(partition dim = sample).  The host converts tops/lefts into {0,1} uint8
row/col masks [512,84]; the device builds the [128, 84*84] patch mask with
a single broadcast tensor_tensor multiply per group, then for each of the
9 channels streams the x tile through SBUF, overwrites the patch with one
copy_predicated (data = per-partition color broadcast), and streams it out.
Everything outside the patch is a bit-exact DMA passthrough of x; inside
the patch the color value is copied bit-exactly, so the result matches the
reference exactly in fp32.
"""

import numpy as np

import concourse.bacc as bacc
import concourse.tile as tile
from concourse import mybir
from concourse.bass_utils import run_bass_kernel_spmd

N_CORES = 8
N, C, H, W = 4096, 9, 84, 84
PATCH = 28
NL = N // N_CORES  # samples per core
P = 128            # SBUF partitions
G = NL // P        # groups per core
HW = H * W

_cached = {}


def build_nc():
    """Build + compile the per-core Bass program (identical on all cores)."""
    nc = bacc.Bacc(
        "TRN2",
        target_bir_lowering=False,
        debug=False,
        num_devices=N_CORES,
    )
    f32 = mybir.dt.float32
    u8 = mybir.dt.uint8
    x = nc.dram_tensor("x", [NL, C * HW], f32, kind="ExternalInput").ap()
    colors = nc.dram_tensor("colors", [NL, C], f32, kind="ExternalInput").ap()
    rmask = nc.dram_tensor("rmask", [NL, H], u8, kind="ExternalInput").ap()
    cmask = nc.dram_tensor("cmask", [NL, W], u8, kind="ExternalInput").ap()
    out = nc.dram_tensor("out", [NL, C * HW], f32, kind="ExternalOutput").ap()

    with tile.TileContext(nc) as tc:
        with (
            tc.tile_pool(name="xp", bufs=6) as xp,
            tc.tile_pool(name="mp", bufs=2) as mp,
            tc.tile_pool(name="sp", bufs=2) as sp,
        ):
            for g in range(G):
                sl = slice(g * P, (g + 1) * P)
                rm = sp.tile([P, H], u8, tag="rm")
                cm = sp.tile([P, W], u8, tag="cm")
                co = sp.tile([P, C], f32, tag="co")
                nc.sync.dma_start(rm[:], rmask[sl, :])
                nc.sync.dma_start(cm[:], cmask[sl, :])
                nc.sync.dma_start(co[:], colors[sl, :])

                # m[p, h*84+w] = rmask[p,h] * cmask[p,w]  (per-sample outer product)
                m = mp.tile([P, HW], u8, tag="m")
                m3 = m[:].rearrange("p (h w) -> p h w", h=H, w=W)
                rm3 = rm[:].unsqueeze(2).broadcast_to((P, H, W))
                cm3 = cm[:].unsqueeze(1).broadcast_to((P, H, W))
                nc.vector.tensor_tensor(m3, rm3, cm3, mybir.AluOpType.mult)

                for c in range(C):
                    xt = xp.tile([P, HW], f32, tag="xt")
                    # loads on the SP HWDGE ring, stores on the ACT HWDGE ring:
                    # a store waiting on its predicated-copy must not stall the
                    # descriptor flow of later loads (FIFO per issuing engine).
                    nc.sync.dma_start(xt[:], x[sl, c * HW:(c + 1) * HW])
                    nc.vector.copy_predicated(
                        xt[:], m[:], co[:, c:c + 1].broadcast_to((P, HW))
                    )
                    nc.scalar.dma_start(out[sl, c * HW:(c + 1) * HW], xt[:])

    nc.compile()
    return nc


def get_nc():
    if "nc" not in _cached:
        _cached["nc"] = build_nc()
    return _cached["nc"]


def make_in_maps(x, colors, tops, lefts):
    """Shard full inputs into per-core input maps (host-side, tiny work)."""
    x = np.ascontiguousarray(x, dtype=np.float32).reshape(N, C * HW)
    colors = np.ascontiguousarray(colors, dtype=np.float32)
    tops = np.asarray(tops).astype(np.int32, copy=False)
    lefts = np.asarray(lefts).astype(np.int32, copy=False)

    rows = np.arange(H, dtype=np.int32)
    cols = np.arange(W, dtype=np.int32)
    rmask = (
        (rows[None, :] >= tops[:, None]) & (rows[None, :] < tops[:, None] + PATCH)
    ).astype(np.uint8)
    cmask = (
        (cols[None, :] >= lefts[:, None]) & (cols[None, :] < lefts[:, None] + PATCH)
    ).astype(np.uint8)

    in_maps = []
    for k in range(N_CORES):
        sl = slice(k * NL, (k + 1) * NL)
        in_maps.append(
            {
                "x": x[sl],
                "colors": colors[sl],
                "rmask": rmask[sl],
                "cmask": cmask[sl],
            }
        )
    return in_maps


def run(in_maps, trace=False, **kwargs):
    nc = get_nc()
    return run_bass_kernel_spmd(
        nc, in_maps, list(range(N_CORES)), trace=trace, **kwargs
    )


def kernel(x, colors, tops, lefts):
    in_maps = make_in_maps(x, colors, tops, lefts)
    res = run(in_maps)
    out = np.concatenate([r["out"] for r in res.results], axis=0)
    return out.reshape(N, C, H, W)


# revision 8
# speedup vs baseline: 1.0091x; 1.0091x over previous
"""CutoutColor Trainium2 kernel.

out[n,c,h,w] = colors[n,c] if (tops[n] <= h < tops[n]+28 and
                               lefts[n] <= w < lefts[n]+28) else x[n,c,h,w]

Strategy: pure data parallel over the batch axis, 512 samples per core on
8 NeuronCores.  On each core, samples are processed in 4 groups of 128
(partition dim = sample).  The host converts tops/lefts into {0,1} uint8
row/col masks [512,84]; the device builds the [128, 84*84] patch mask with
a single broadcast tensor_tensor multiply per group, then for each of the
9 channels streams the x tile through SBUF, overwrites the patch with one
copy_predicated (data = per-partition color broadcast), and streams it out.
Everything outside the patch is a bit-exact DMA passthrough of x; inside
the patch the color value is copied bit-exactly, so the result matches the
reference exactly in fp32.
"""

import numpy as np

import concourse.bacc as bacc
import concourse.tile as tile
from concourse import mybir
from concourse.bass_utils import run_bass_kernel_spmd

N_CORES = 8
N, C, H, W = 4096, 9, 84, 84
PATCH = 28
NL = N // N_CORES  # samples per core
P = 128            # SBUF partitions
G = NL // P        # groups per core
HW = H * W

_cached = {}


def build_nc():
    """Build + compile the per-core Bass program (identical on all cores)."""
    nc = bacc.Bacc(
        "TRN2",
        target_bir_lowering=False,
        debug=False,
        num_devices=N_CORES,
    )
    f32 = mybir.dt.float32
    u8 = mybir.dt.uint8
    x = nc.dram_tensor("x", [NL, C * HW], f32, kind="ExternalInput").ap()
    colors = nc.dram_tensor("colors", [NL, C], f32, kind="ExternalInput").ap()
    rmask = nc.dram_tensor("rmask", [NL, H], u8, kind="ExternalInput").ap()
    cmask = nc.dram_tensor("cmask", [NL, W], u8, kind="ExternalInput").ap()
    out = nc.dram_tensor("out", [NL, C * HW], f32, kind="ExternalOutput").ap()

    with tile.TileContext(nc) as tc:
        with (
            tc.tile_pool(name="xp", bufs=5) as xp,
            tc.tile_pool(name="mp", bufs=G) as mp,
            tc.tile_pool(name="sp", bufs=G) as sp,
        ):
            # The machine is write-throughput bound (~200 GB/s/core HBM
            # writes), so stores must never starve.  Build ALL group masks up
            # front: a mask built lazily queues on DVE behind the previous
            # group's predicated copies, which stalls the new group's first
            # stores (and, via buffer slots, the loads) at every boundary.
            ms, cos = [], []
            for g in range(G):
                sl = slice(g * P, (g + 1) * P)
                rm = sp.tile([P, H], u8, tag="rm")
                cm = sp.tile([P, W], u8, tag="cm")
                co = sp.tile([P, C], f32, tag="co")
                # tiny loads on the store (ACT) ring: it is empty at start,
                # and this keeps the load (SP) ring free for x tiles.
                nc.scalar.dma_start(rm[:], rmask[sl, :])
                nc.scalar.dma_start(cm[:], cmask[sl, :])
                nc.scalar.dma_start(co[:], colors[sl, :])
                # m[p, h*84+w] = rmask[p,h] * cmask[p,w] (per-sample outer product)
                m = mp.tile([P, HW], u8, tag="m")
                m3 = m[:].rearrange("p (h w) -> p h w", h=H, w=W)
                rm3 = rm[:].unsqueeze(2).broadcast_to((P, H, W))
                cm3 = cm[:].unsqueeze(1).broadcast_to((P, H, W))
                nc.vector.tensor_tensor(m3, rm3, cm3, mybir.AluOpType.mult)
                ms.append(m)
                cos.append(co)

            for g in range(G):
                sl = slice(g * P, (g + 1) * P)
                m, co = ms[g], cos[g]
                for c in range(C):
                    xt = xp.tile([P, HW], f32, tag="xt")
                    # loads on the SP HWDGE ring, stores on the ACT HWDGE ring:
                    # a store waiting on its predicated-copy must not stall the
                    # descriptor flow of later loads (FIFO per issuing engine).
                    nc.sync.dma_start(xt[:], x[sl, c * HW:(c + 1) * HW])
                    nc.vector.copy_predicated(
                        xt[:], m[:], co[:, c:c + 1].broadcast_to((P, HW))
                    )
                    nc.scalar.dma_start(out[sl, c * HW:(c + 1) * HW], xt[:])

    nc.compile()
    return nc


def get_nc():
    if "nc" not in _cached:
        _cached["nc"] = build_nc()
    return _cached["nc"]


def make_in_maps(x, colors, tops, lefts):
    """Shard full inputs into per-core input maps (host-side, tiny work)."""
    x = np.ascontiguousarray(x, dtype=np.float32).reshape(N, C * HW)
    colors = np.ascontiguousarray(colors, dtype=np.float32)
    tops = np.asarray(tops).astype(np.int32, copy=False)
    lefts = np.asarray(lefts).astype(np.int32, copy=False)

    rows = np.arange(H, dtype=np.int32)
    cols = np.arange(W, dtype=np.int32)
    rmask = (
        (rows[None, :] >= tops[:, None]) & (rows[None, :] < tops[:, None] + PATCH)
    ).astype(np.uint8)
    cmask = (
        (cols[None, :] >= lefts[:, None]) & (cols[None, :] < lefts[:, None] + PATCH)
    ).astype(np.uint8)

    in_maps = []
    for k in range(N_CORES):
        sl = slice(k * NL, (k + 1) * NL)
        in_maps.append(
            {
                "x": x[sl],
                "colors": colors[sl],
                "rmask": rmask[sl],
                "cmask": cmask[sl],
            }
        )
    return in_maps


def run(in_maps, trace=False, **kwargs):
    nc = get_nc()
    return run_bass_kernel_spmd(
        nc, in_maps, list(range(N_CORES)), trace=trace, **kwargs
    )


def kernel(x, colors, tops, lefts):
    in_maps = make_in_maps(x, colors, tops, lefts)
    res = run(in_maps)
    out = np.concatenate([r["out"] for r in res.results], axis=0)
    return out.reshape(N, C, H, W)
